# revision 4
# baseline (speedup 1.0000x reference)
"""Trainium2 Bass kernel for nn_Attention_86199993631321.

Reference computation (B=8, N=128, H=512):
    pair[b,i,j,:] = x[b,i,:] + x[b,j,:]
    out = pair @ W.T + b                # [B, N, N, H]

Key algebraic simplification: the Linear is applied to a *sum*, so
    out[b,i,j,:] = P[b,i,:] + P[b,j,:]   where P = x @ W.T + 0.5*b
This turns 68.7 GFLOP of einsum into a 0.5 GFLOP matmul plus a broadcast-add
that only has to *write* the 268 MB output.

Sharding: pure data-parallel over batch B (core b handles batch b), no
collectives.  Per core:
  - P = x_b @ W.T   via TensorE (inputs pre-transposed on host, packed into a
    single DRAM tensor so the first matmul needs only one semaphore wait —
    fp32 Matmult instructions can carry at most ONE sync wait in walrus).
  - P_rep = P + 0.5*b (bias comes in via a DMA partition-broadcast tile),
    replicated 4x along the free dim for later tensor_tensor reads.
  - P's rows are staged to partition 0 (small SBUF->SBUF DMAs) so they can be
    the moving operand of K=1 rank-1 matmuls: bcast_j = ones ⊗ P[j,:]
    (PE broadcasts a row across all 128 partitions into PSUM).
  - VectorE adds P_rep + bcast (PSUM) -> SBUF out tiles.
  - Out tiles [128, 8*512] f32 (2 MB) DMA to HBM; each partition writes one
    contiguous 16 KB run.
A 1x1 "absorber" matmul first-writes each PSUM tile so the PSUM-slot-reuse
wait lands on it, keeping every real matmul at <=1 sync wait.
"""

import sys

if "/opt/trn_rl_repo" not in sys.path:
    sys.path.insert(0, "/opt/trn_rl_repo")

import numpy as np

B, N, H = 8, 128, 512
NCORES = 8
KC = H // 128  # contraction chunks for the P matmul
JBLK = 8       # j rows per output tile -> [128, JBLK*H] f32 = 2 MB per DMA
TTW = 4        # j rows per PSUM tile / tensor_tensor op ([128, TTW*H] = 4 banks)
# packed input layout (per core): wx[h, 0:128] = x.T, wx[h, 128:640] = W.T,
# wx[h, 640:768] = 1.0
WXW = N + H + 128

_BUILT = {}


def _build_nc():
    import concourse.bass as bass
    import concourse.bacc as bacc
    import concourse.tile as tile
    from concourse import mybir

    f32 = mybir.dt.float32

    nc = bacc.Bacc()
    wx_ext = nc.declare_dram_parameter("wx", [H, WXW], f32, isOutput=False)
    hb_ext = nc.declare_dram_parameter("halfb", [1, H], f32, isOutput=False)
    out_ext = nc.declare_dram_parameter("out", [N, N, H], f32, isOutput=True)

    with tile.TileContext(nc) as tc:
        with (
            tc.tile_pool(name="const", bufs=1) as const,
            tc.tile_pool(name="stage", bufs=2) as stage,
            tc.tile_pool(name="outp", bufs=3) as outp,
            tc.tile_pool(name="psum", bufs=2, space="PSUM") as psum,
        ):
            # ---- load packed inputs: one DMA -> one wait for the matmuls ----
            wx_sb = const.tile([128, KC, WXW], f32)  # [h_local, (kc, m)]
            nc.sync.dma_start(
                out=wx_sb, in_=wx_ext.rearrange("(c p) m -> p c m", p=128)
            )
            ones_sb = wx_sb[0:1, 0, N + H : N + H + 128]  # [1, 128] @ partition 0

            # 0.5*b broadcast to all 128 partitions via DMA (groupnorm pattern)
            hb_bc = const.tile([128, H], f32)
            hb_ap = hb_ext[:, :]
            hb_bcast_src = bass.AP(
                tensor=hb_ap.tensor,
                offset=hb_ap.offset,
                ap=[[0, 128], list(hb_ap.ap[-1])],
            )
            nc.gpsimd.dma_start(out=hb_bc, in_=hb_bcast_src)

            # ---- P = x @ W.T -> PSUM [128(i), 512(o)] ----
            ps_proj = psum.tile([128, TTW * H], f32, tag="ps")
            for c in range(KC):
                nc.tensor.matmul(
                    ps_proj[:, 0:H],
                    wx_sb[:, c, 0:N],
                    wx_sb[:, c, N : N + H],
                    start=(c == 0),
                    stop=(c == KC - 1),
                )

            # P_rep = P + 0.5*b, replicated TTW times along the free dim.
            P_rep = const.tile([128, TTW, H], f32)
            for u in range(TTW):
                nc.vector.tensor_tensor(
                    out=P_rep[:, u, :],
                    in0=ps_proj[:, 0:H],
                    in1=hb_bc,
                    op=mybir.AluOpType.add,
                )

            # ---- main loop over j-blocks ----
            for jt in range(N // JBLK):
                # stage P rows j0..j0+JBLK to partition 0 for rank-1 matmuls
                chunk = stage.tile([1, JBLK * H], f32)
                nc.sync.dma_start(
                    out=chunk,
                    in_=P_rep[jt * JBLK : (jt + 1) * JBLK, 0, :],
                )
                out_tile = outp.tile([128, JBLK, H], f32)
                for s in range(JBLK // TTW):
                    ps = psum.tile([128, TTW * H], f32, tag="ps")
                    # absorber: first-writes the psum tile so the slot-reuse
                    # (WAR vs the TT two groups back) wait lands here, leaving
                    # the real matmuls with at most one wait each.
                    nc.tensor.matmul(
                        ps[0:1, 0:1],
                        ones_sb[0:1, 0:1],
                        ones_sb[0:1, 0:1],
                        start=True,
                        stop=True,
                    )
                    for u in range(TTW):
                        jj = s * TTW + u
                        nc.tensor.matmul(
                            ps[:, u * H : (u + 1) * H],
                            ones_sb,
                            chunk[0:1, jj * H : (jj + 1) * H],
                            start=True,
                            stop=True,
                        )
                    nc.vector.tensor_tensor(
                        out=out_tile[:, s * TTW : (s + 1) * TTW, :],
                        in0=P_rep[:, :, :],
                        in1=ps.rearrange("p (u h) -> p u h", u=TTW),
                        op=mybir.AluOpType.add,
                    )
                nc.sync.dma_start(
                    out=out_ext[:, jt * JBLK : (jt + 1) * JBLK, :],
                    in_=out_tile,
                )
    nc.compile()
    return nc


def _get_nc():
    if "nc" not in _BUILT:
        _BUILT["nc"] = _build_nc()
    return _BUILT["nc"]


def _make_in_maps(local_feats, W, b):
    local_feats = np.asarray(local_feats, dtype=np.float32)
    W = np.asarray(W, dtype=np.float32)
    b = np.asarray(b, dtype=np.float32)
    hb = np.ascontiguousarray((0.5 * b).reshape(1, H))
    base = np.empty((H, WXW), dtype=np.float32)
    base[:, N : N + H] = W.T
    base[:, N + H :] = 1.0
    in_maps = []
    for c in range(NCORES):
        wx = base.copy()
        wx[:, :N] = local_feats[c].T
        in_maps.append({"wx": wx, "halfb": hb})
    return in_maps


def kernel(local_feats, W, b):
    from concourse.bass_utils import run_bass_kernel_spmd

    nc = _get_nc()
    in_maps = _make_in_maps(local_feats, W, b)
    res = run_bass_kernel_spmd(nc, in_maps, core_ids=list(range(NCORES)))
    out = np.stack([res.results[c]["out"] for c in range(NCORES)], axis=0)
    return out


def run_profiled(local_feats, W, b, **trace_kwargs):
    """Like kernel() but with neuron-profile tracing; returns (out, results)."""
    from concourse.bass_utils import run_bass_kernel_spmd

    nc = _get_nc()
    in_maps = _make_in_maps(local_feats, W, b)
    res = run_bass_kernel_spmd(
        nc, in_maps, core_ids=list(range(NCORES)), trace=True, **trace_kwargs
    )
    out = np.stack([res.results[c]["out"] for c in range(NCORES)], axis=0)
    return out, res


# revision 8
# speedup vs baseline: 2.7040x; 2.7040x over previous
"""Trainium2 Bass kernel for nn_Attention_86199993631321.

Reference computation (B=8, N=128, H=512):
    pair[b,i,j,:] = x[b,i,:] + x[b,j,:]
    out = pair @ W.T + b                # [B, N, N, H]

Key algebraic simplification: the Linear is applied to a *sum*, so
    out[b,i,j,:] = P[b,i,:] + P[b,j,:]   where P = x @ W.T + 0.5*b
This turns 68.7 GFLOP of einsum into a 0.5 GFLOP matmul plus a broadcast-add
that only has to *write* the 268 MB output.

Sharding: pure data-parallel over batch B (core b handles batch b), no
collectives.  Per core:
  - P = x_b @ W.T   via TensorE (inputs pre-transposed on host, packed into a
    single DRAM tensor so the first matmul needs only one semaphore wait —
    fp32 Matmult instructions can carry at most ONE sync wait in walrus).
  - P_rep = P + 0.5*b (bias comes in via a DMA partition-broadcast tile),
    replicated 4x along the free dim for later tensor_tensor reads.
  - P's rows are staged to partition 0 (small SBUF->SBUF DMAs) so they can be
    the moving operand of K=1 rank-1 matmuls: bcast_j = ones ⊗ P[j,:]
    (PE broadcasts a row across all 128 partitions into PSUM).
  - VectorE adds P_rep + bcast (PSUM) -> SBUF out tiles.
  - Out tiles [128, 8*512] f32 (2 MB) DMA to HBM; each partition writes one
    contiguous 16 KB run.
A 1x1 "absorber" matmul first-writes each PSUM tile so the PSUM-slot-reuse
wait lands on it, keeping every real matmul at <=1 sync wait.
"""

import sys

if "/opt/trn_rl_repo" not in sys.path:
    sys.path.insert(0, "/opt/trn_rl_repo")

import numpy as np

B, N, H = 8, 128, 512
NCORES = 8
KC = H // 128  # contraction chunks for the P matmul
JBLK = 8       # j rows per output tile -> [128, JBLK*H] f32 = 2 MB per DMA
TTW = 4        # j rows per PSUM tile / tensor_tensor op ([128, TTW*H] = 4 banks)
# packed input layout (per core): wx[h, 0:128] = x.T, wx[h, 128:640] = W.T
WXW = N + H

_BUILT = {}


def _build_nc():
    import concourse.bass as bass
    import concourse.bacc as bacc
    import concourse.tile as tile
    from concourse import mybir

    f32 = mybir.dt.float32
    bf16 = mybir.dt.bfloat16

    nc = bacc.Bacc()
    wx_ext = nc.declare_dram_parameter("wx", [H, WXW], f32, isOutput=False)
    hb_ext = nc.declare_dram_parameter("halfb", [1, H], f32, isOutput=False)
    out_ext = nc.declare_dram_parameter("out", [N, N, H], f32, isOutput=True)

    with tile.TileContext(nc) as tc:
        with (
            tc.tile_pool(name="const", bufs=1) as const,
            tc.tile_pool(name="stage", bufs=2) as stage,
            tc.tile_pool(name="outp", bufs=3) as outp,
            tc.tile_pool(name="psum", bufs=2, space="PSUM") as psum,
        ):
            # ---- load packed inputs: one DMA -> one wait for the matmuls ----
            wx_sb = const.tile([128, KC, WXW], f32)  # [h_local, (kc, m)]
            nc.sync.dma_start(
                out=wx_sb, in_=wx_ext.rearrange("(c p) m -> p c m", p=128)
            )
            # bf16 ones row for the rank-1 broadcast matmuls (bf16 -> single
            # PE pass; fp32 matmuls cost 2 passes)
            ones_sb = const.tile([1, 128], bf16)
            nc.vector.memset(ones_sb, 1.0)

            # 0.5*b broadcast to all 128 partitions via DMA (groupnorm pattern)
            hb_bc = const.tile([128, H], f32)
            hb_ap = hb_ext[:, :]
            hb_bcast_src = bass.AP(
                tensor=hb_ap.tensor,
                offset=hb_ap.offset,
                ap=[[0, 128], list(hb_ap.ap[-1])],
            )
            nc.gpsimd.dma_start(out=hb_bc, in_=hb_bcast_src)

            # ---- P = x @ W.T -> PSUM [128(i), 512(o)] ----
            ps_proj = psum.tile([128, TTW * H], f32, tag="ps")
            for c in range(KC):
                nc.tensor.matmul(
                    ps_proj[:, 0:H],
                    wx_sb[:, c, 0:N],
                    wx_sb[:, c, N : N + H],
                    start=(c == 0),
                    stop=(c == KC - 1),
                )

            # P_rep = P + 0.5*b, replicated TTW times along the free dim.
            P_rep = const.tile([128, TTW, H], f32)
            for u in range(TTW):
                nc.vector.tensor_tensor(
                    out=P_rep[:, u, :],
                    in0=ps_proj[:, 0:H],
                    in1=hb_bc,
                    op=mybir.AluOpType.add,
                )

            # ---- main loop over j-blocks ----
            for jt in range(N // JBLK):
                # stage P rows j0..j0+JBLK to partition 0 for rank-1 matmuls
                # (gpsimd DMA casts f32 -> bf16 on the way)
                chunk = stage.tile([1, JBLK * H], bf16)
                nc.gpsimd.dma_start(
                    out=chunk,
                    in_=P_rep[jt * JBLK : (jt + 1) * JBLK, 0, :],
                )
                out_tile = outp.tile([128, JBLK, H], f32)
                for s in range(JBLK // TTW):
                    ps = psum.tile([128, TTW * H], f32, tag="ps")
                    for u in range(TTW):
                        jj = s * TTW + u
                        nc.tensor.matmul(
                            ps[:, u * H : (u + 1) * H],
                            ones_sb,
                            chunk[0:1, jj * H : (jj + 1) * H],
                            start=True,
                            stop=True,
                        )
                    nc.vector.tensor_tensor(
                        out=out_tile[:, s * TTW : (s + 1) * TTW, :],
                        in0=P_rep[:, :, :],
                        in1=ps.rearrange("p (u h) -> p u h", u=TTW),
                        op=mybir.AluOpType.add,
                    )
                nc.sync.dma_start(
                    out=out_ext[:, jt * JBLK : (jt + 1) * JBLK, :],
                    in_=out_tile,
                )
    nc.compile()
    return nc


def _get_nc():
    if "nc" not in _BUILT:
        _BUILT["nc"] = _build_nc()
    return _BUILT["nc"]


def _make_in_maps(local_feats, W, b):
    local_feats = np.asarray(local_feats, dtype=np.float32)
    W = np.asarray(W, dtype=np.float32)
    b = np.asarray(b, dtype=np.float32)
    hb = np.ascontiguousarray((0.5 * b).reshape(1, H))
    base = np.empty((H, WXW), dtype=np.float32)
    base[:, N : N + H] = W.T
    base[:, N + H :] = 1.0
    in_maps = []
    for c in range(NCORES):
        wx = base.copy()
        wx[:, :N] = local_feats[c].T
        in_maps.append({"wx": wx, "halfb": hb})
    return in_maps


def kernel(local_feats, W, b):
    from concourse.bass_utils import run_bass_kernel_spmd

    nc = _get_nc()
    in_maps = _make_in_maps(local_feats, W, b)
    res = run_bass_kernel_spmd(nc, in_maps, core_ids=list(range(NCORES)))
    out = np.stack([res.results[c]["out"] for c in range(NCORES)], axis=0)
    return out


def run_profiled(local_feats, W, b, **trace_kwargs):
    """Like kernel() but with neuron-profile tracing; returns (out, results)."""
    from concourse.bass_utils import run_bass_kernel_spmd

    nc = _get_nc()
    in_maps = _make_in_maps(local_feats, W, b)
    res = run_bass_kernel_spmd(
        nc, in_maps, core_ids=list(range(NCORES)), trace=True, **trace_kwargs
    )
    out = np.stack([res.results[c]["out"] for c in range(NCORES)], axis=0)
    return out, res


# revision 14
# speedup vs baseline: 3.0255x; 1.1189x over previous
"""Trainium2 Bass kernel for nn_Attention_86199993631321.

Reference computation (B=8, N=128, H=512):
    pair[b,i,j,:] = x[b,i,:] + x[b,j,:]
    out = pair @ W.T + b                # [B, N, N, H]

Key algebraic simplification: the Linear is applied to a *sum*, so
    out[b,i,j,:] = P[b,i,:] + P[b,j,:]   where P = x @ W.T + 0.5*b
This turns 68.7 GFLOP of einsum into a 0.5 GFLOP matmul plus a broadcast-add
that only has to *write* the 268 MB output.

Sharding: pure data-parallel over batch B (core b handles batch b), no
collectives.  Per core:
  - P = x_b @ W.T  via TensorE (inputs pre-transposed on host, packed into a
    single DRAM tensor).
  - P_rep = P + 0.5*b (bias via a DMA partition-broadcast tile), replicated
    4x along the free dim for later tensor_tensor reads.
  - P's rows are staged (cast to bf16) into the four legal PE row-group base
    partitions {0,32,64,96}; K=1 rank-1 matmuls ones ⊗ P[j,:] then run 4-way
    concurrently in the PE array via tile_position row groups.
  - VectorE adds P_rep + bcast (PSUM) -> bf16 SBUF out tiles.
  - Out tiles go to HBM as bf16 (half the write traffic); the host upcasts
    to f32.  The row permutation introduced by the quadrant packing is
    undone by the DMA access pattern (i (q s) o -> i (s q) o).
"""

import sys

if "/opt/trn_rl_repo" not in sys.path:
    sys.path.insert(0, "/opt/trn_rl_repo")

import numpy as np

B, N, H = 8, 128, 512
NCORES = 8
KC = H // 128  # contraction chunks for the P matmul
JBLK = 8       # j rows per output tile
TTW = 4        # j rows per PSUM tile / tensor_tensor op ([128, TTW*H] = 4 banks)
NQ = 4         # PE row-group quadrants
RPQ = JBLK // NQ  # rows per quadrant in a chunk (2)
# packed input layout (per core): wx[h, 0:128] = x.T, wx[h, 128:640] = W.T
WXW = N + H

_BUILT = {}


def _build_nc():
    import concourse.bass as bass
    import concourse.bacc as bacc
    import concourse.tile as tile
    from concourse import mybir

    f32 = mybir.dt.float32
    bf16 = mybir.dt.bfloat16

    nc = bacc.Bacc()
    wx_ext = nc.declare_dram_parameter("wx", [H, WXW], f32, isOutput=False)
    hb_ext = nc.declare_dram_parameter("halfb", [1, H], f32, isOutput=False)
    out_ext = nc.declare_dram_parameter("out", [N, N, H], bf16, isOutput=True)

    with tile.TileContext(nc) as tc:
        with (
            tc.tile_pool(name="const", bufs=1) as const,
            tc.tile_pool(name="stage", bufs=2) as stage,
            tc.tile_pool(name="outp", bufs=3) as outp,
            tc.tile_pool(name="psum", bufs=2, space="PSUM") as psum,
        ):
            # ---- load packed inputs ----
            wx_sb = const.tile([128, KC, WXW], f32)  # [h_local, (kc, m)]
            nc.sync.dma_start(
                out=wx_sb, in_=wx_ext.rearrange("(c p) m -> p c m", p=128)
            )
            # bf16 ones; slices at partitions {0,32,64,96} feed the four
            # concurrent row-group matmuls.
            ones_sb = const.tile([128, 128], bf16)
            nc.vector.memset(ones_sb, 1.0)

            # 0.5*b broadcast to all 128 partitions via DMA
            hb_bc = const.tile([128, H], f32)
            hb_ap = hb_ext[:, :]
            hb_bcast_src = bass.AP(
                tensor=hb_ap.tensor,
                offset=hb_ap.offset,
                ap=[[0, 128], list(hb_ap.ap[-1])],
            )
            nc.gpsimd.dma_start(out=hb_bc, in_=hb_bcast_src)

            # ---- P = x @ W.T -> PSUM [128(i), 512(o)] ----
            ps_proj = psum.tile([128, TTW * H], f32, tag="ps")
            for c in range(KC):
                nc.tensor.matmul(
                    ps_proj[:, 0:H],
                    wx_sb[:, c, 0:N],
                    wx_sb[:, c, N : N + H],
                    start=(c == 0),
                    stop=(c == KC - 1),
                )

            # P_rep = P + 0.5*b (f32), replicated TTW times along the free dim.
            P_rep = const.tile([128, TTW, H], f32)
            for u in range(TTW):
                nc.vector.tensor_tensor(
                    out=P_rep[:, u, :],
                    in0=ps_proj[:, 0:H],
                    in1=hb_bc,
                    op=mybir.AluOpType.add,
                )

            # ---- main loop over j-blocks ----
            for jt in range(N // JBLK):
                j0 = jt * JBLK
                # stage P rows into quadrants (bf16): partition 32*q holds
                # rows j0+2q, j0+2q+1 as [1, RPQ*H]
                chunk = stage.tile([128, RPQ * H], bf16)
                for q in range(NQ):
                    nc.gpsimd.dma_start(
                        out=chunk[q * 32 : q * 32 + 1, :],
                        in_=P_rep[j0 + RPQ * q : j0 + RPQ * (q + 1), 0, :],
                    )
                out_tile = outp.tile([128, JBLK, H], bf16)
                # psum tile t covers rows j0+4t .. j0+4t+3 (quadrants 2t,2t+1,
                # rows stay in natural order).  Matmuls are issued q-fastest so
                # all four PE row groups overlap in the array.
                ps_a = psum.tile([128, TTW * H], f32, tag="ps")
                ps_b = psum.tile([128, TTW * H], f32, tag="ps")
                ps_tiles = [ps_a, ps_b]
                for s in range(RPQ):
                    for q in range(NQ):
                        ps_t = ps_tiles[q // 2]
                        slot = (q % 2) * RPQ + s  # local row within the tile
                        nc.tensor.matmul(
                            ps_t[:, slot * H : (slot + 1) * H],
                            ones_sb[q * 32 : q * 32 + 1, :],
                            chunk[q * 32 : q * 32 + 1, s * H : (s + 1) * H],
                            start=True,
                            stop=True,
                            tile_position=(q * 32, 0),
                        )
                for t, ps_t in enumerate(ps_tiles):
                    nc.vector.tensor_tensor(
                        out=out_tile[:, t * TTW : (t + 1) * TTW, :],
                        in0=P_rep[:, :, :],
                        in1=ps_t.rearrange("p (u h) -> p u h", u=TTW),
                        op=mybir.AluOpType.add,
                    )
                nc.sync.dma_start(
                    out=out_ext[:, j0 : j0 + JBLK, :], in_=out_tile
                )
    nc.compile()
    return nc


def _get_nc():
    if "nc" not in _BUILT:
        _BUILT["nc"] = _build_nc()
    return _BUILT["nc"]


def _make_in_maps(local_feats, W, b):
    local_feats = np.asarray(local_feats, dtype=np.float32)
    W = np.asarray(W, dtype=np.float32)
    b = np.asarray(b, dtype=np.float32)
    hb = np.ascontiguousarray((0.5 * b).reshape(1, H))
    base = np.empty((H, WXW), dtype=np.float32)
    base[:, N : N + H] = W.T
    in_maps = []
    for c in range(NCORES):
        wx = base.copy()
        wx[:, :N] = local_feats[c].T
        in_maps.append({"wx": wx, "halfb": hb})
    return in_maps


def _collect(res):
    return np.stack(
        [np.asarray(res.results[c]["out"]).astype(np.float32) for c in range(NCORES)],
        axis=0,
    )


def kernel(local_feats, W, b):
    from concourse.bass_utils import run_bass_kernel_spmd

    nc = _get_nc()
    in_maps = _make_in_maps(local_feats, W, b)
    res = run_bass_kernel_spmd(nc, in_maps, core_ids=list(range(NCORES)))
    return _collect(res)


def run_profiled(local_feats, W, b, **trace_kwargs):
    """Like kernel() but with neuron-profile tracing; returns (out, results)."""
    from concourse.bass_utils import run_bass_kernel_spmd

    nc = _get_nc()
    in_maps = _make_in_maps(local_feats, W, b)
    res = run_bass_kernel_spmd(
        nc, in_maps, core_ids=list(range(NCORES)), trace=True, **trace_kwargs
    )
    return _collect(res), res
